# revision 41
# baseline (speedup 1.0000x reference)
"""CavAttention Trainium2 kernel (bf16 datapath rewrite).

Computation (per spatial location (b,h,w), L=5 "cav" slots, 8 heads x 32 dim):
  qkv = x @ w_qkv ; att = softmax_j(mask * q_i.k_j / sqrt(d)) ; o = att @ v ; out = o @ w_out + b_out

Distribution: shard the H axis (48) across the 8 cores (6 each); weights replicated.

Per-core layout: locations (b,h,w) ride the 128 SBUF partitions, (l, head, d)
rides the free axis.  Everything movable runs in bf16 (rel-err budget 2e-2,
measured 4.4e-3 on CPU sim): bf16 PE stationaries get FWL (27ns weight loads),
bf16 DVE tensor_tensor ops run in 2x packed mode.  Reductions over d (QK) and
j (AV) are pairwise trees of 2x TT-adds instead of 1x-mode monolithic reduces.
Softmax skips max-subtraction (logits bounded by ~18; exp stays in fp32 range)
and folds the mask in as a -1e4 logit bias.  PSUM->SBUF copies are batched.
"""

import numpy as np

B, L, H, W, C = 2, 5, 48, 176, 256
HEADS, DIM_HEAD = 8, 32
INNER = HEADS * DIM_HEAD  # 256
SCALE = DIM_HEAD ** -0.5
NCORES = 8
HP = H // NCORES  # 6 h-planes per core
NBH = B * HP      # 12 (b,h) blocks per core
LOCS = NBH * W    # 2112 locations per core
PTILE = 128       # locations per tile
NTILES = (LOCS + PTILE - 1) // PTILE  # 17

# engine split knobs: which i-slices of the attention core run on gpsimd.
# gpsimd shares its SBUF read port with the DVE, so offloading slows DVE
# tensor_tensor ops ~25-30% — measured net-negative.  Keep everything on DVE.
AV_MUL_POOL_I = ()
QK_MUL_POOL_I = ()

_cached = {}


def _pieces(s, e):
    """Split flat loc range [s,e) into (p0, b, h, w0, w1) pieces within (b,h) blocks."""
    out = []
    cur = s
    while cur < e:
        bh = cur // W
        w0 = cur % W
        w1 = min(W, w0 + (e - cur))
        out.append((cur - s, bh // HP, bh % HP, w0, w1))
        cur += w1 - w0
    return out


def _build_bass():
    import concourse.bass as bass
    import concourse.bacc as bacc
    import concourse.tile as tile
    from concourse import mybir
    from concourse.masks import make_identity

    f32 = mybir.dt.float32
    bf16 = mybir.dt.bfloat16
    i32 = mybir.dt.int32
    Alu = mybir.AluOpType

    nc = bacc.Bacc()
    # x arrives pre-transposed and pre-cast on the host: [cc, c, b, h, l, w]
    xT_d = nc.dram_tensor("xT", [2, 128, B, HP, L, W], bf16, kind="ExternalInput")
    # mask arrives as a pre-computed f32 logit bias (0 valid / -1e4 masked)
    mb_d = nc.dram_tensor("mbias", [B, HP, W, L], f32, kind="ExternalInput")
    wqkv_d = nc.dram_tensor("w_qkv", [C, 3 * INNER], f32, kind="ExternalInput")
    wout_d = nc.dram_tensor("w_out", [INNER, C], f32, kind="ExternalInput")
    bout_d = nc.dram_tensor("b_out", [C], f32, kind="ExternalInput")
    out_d = nc.dram_tensor("out", [B, L, HP, W, C], f32, kind="ExternalOutput")

    with tile.TileContext(nc) as tc:
        with (
            tc.tile_pool(name="singles", bufs=1) as singles,
            tc.tile_pool(name="work", bufs=3) as work,
            tc.tile_pool(name="peri", bufs=3) as peri,
            tc.tile_pool(name="ps_t", bufs=1, space="PSUM") as ps_t,
            tc.tile_pool(name="ps_d", bufs=1, space="PSUM") as ps_d,
            tc.tile_pool(name="ps_qkv", bufs=2, space="PSUM") as ps_qkv,
            tc.tile_pool(name="ps_o", bufs=2, space="PSUM") as ps_o,
        ):
            # ---- constants.  Every tensor a PE matmul touches is produced by
            #      ONE engine (ACT): PE instructions can carry only a single
            #      semaphore wait (walrus S3_LW limit). ----
            ident_l = singles.tile([128, 128], f32)
            make_identity(nc, ident_l)  # gpsimd
            ones_l = singles.tile([1, 128], f32)
            nc.gpsimd.memset(ones_l, 1.0)
            wqkv_l = singles.tile([128, 2, 3 * INNER], f32)
            for cc in range(2):
                nc.sync.dma_start(out=wqkv_l[:, cc, :], in_=wqkv_d[cc * 128:(cc + 1) * 128, :])
            wout_l = singles.tile([128, 2, C], f32)
            for cc in range(2):
                nc.sync.dma_start(out=wout_l[:, cc, :], in_=wout_d[cc * 128:(cc + 1) * 128, :])
            bout_l = singles.tile([1, C], f32)
            nc.sync.dma_start(out=bout_l, in_=bout_d[:].unsqueeze(0))

            ident = singles.tile([128, 128], bf16)
            nc.scalar.copy(out=ident, in_=ident_l)
            ones1 = singles.tile([1, 128], bf16)
            nc.scalar.copy(out=ones1, in_=ones_l)
            wqkv_sb = singles.tile([128, 2, 3 * INNER], bf16)
            nc.scalar.copy(out=wqkv_sb, in_=wqkv_l)
            wout_sb = singles.tile([128, 2, C], bf16)
            nc.scalar.copy(out=wout_sb, in_=wout_l)
            bout_sb = singles.tile([1, C], bf16)
            nc.scalar.copy(out=bout_sb, in_=bout_l)

            def load(t):
                """DMA in (x already transposed+bf16 on host)."""
                s = t * PTILE
                e = min(s + PTILE, LOCS)
                P = e - s
                pieces = _pieces(s, e)

                xt = work.tile([128, 2, L, 128], bf16, tag="xt")
                for (p0, b, h, w0, w1) in pieces:
                    for cc in range(2):
                        nc.sync.dma_start(
                            out=xt[:, cc, :, p0:p0 + (w1 - w0)],
                            in_=xT_d[cc, :, b, h, :, w0:w1],
                        )
                mbias = work.tile([128, L], f32, tag="mbias")
                for (p0, b, h, w0, w1) in pieces:
                    nc.sync.dma_start(
                        out=mbias[p0:p0 + (w1 - w0), :],
                        in_=mb_d[b, h, w0:w1, :],
                    )
                return dict(P=P, pieces=pieces, xt=xt, mbias=mbias)

            def proj(st):
                """qkv projection on PE + ACT psum->sbuf cast."""
                P = st["P"]
                xt = st["xt"]
                qkv_bf = work.tile([128, L, 3 * INNER], bf16, tag="qkv_bf")
                for l in range(L):
                    pq = ps_qkv.tile([128, 3 * INNER], f32, tag="psq")
                    for cc in range(2):
                        for (n0, n1) in ((0, 512), (512, 768)):
                            nc.tensor.matmul(
                                out=pq[:P, n0:n1],
                                lhsT=xt[:, cc, l, :P],
                                rhs=wqkv_sb[:, cc, n0:n1],
                                start=(cc == 0),
                                stop=(cc == 1),
                            )
                    nc.scalar.copy(out=qkv_bf[:P, l, :], in_=pq[:P, :])
                st["qkv_bf"] = qkv_bf

            def warm(src_ap):
                """Tiny matmul on a just-produced DVE tensor: keeps the PE's
                HAM activity monitor seeing matmul work through the attention
                window so the clock gate stays at 8/8 (2.4 GHz)."""
                pd = ps_d.tile([1, 64], f32, tag="dummy")
                nc.tensor.matmul(
                    out=pd[:, 0:src_ap.free_size()], lhsT=ones1[0:1, 0:1],
                    rhs=src_ap, start=True, stop=True, skip_group_check=True,
                )

            def stage_a(st):
                """QK + softmax on DVE/GPSIMD/ACT -> st['pw']."""
                P = st["P"]
                qkv_bf = st["qkv_bf"]
                q_v = qkv_bf[:P, :, 0:INNER].rearrange("p l (m d) -> p l m d", d=DIM_HEAD)
                k_v = qkv_bf[:P, :, INNER:2 * INNER].rearrange("p l (m d) -> p l m d", d=DIM_HEAD)

                def qk_mul(i, eng, tag):
                    qk = peri.tile([128, L, HEADS, DIM_HEAD], bf16, tag=tag)
                    eng.tensor_mul(
                        qk[:P],
                        q_v[:, i:i + 1, :, :].broadcast_to([P, L, HEADS, DIM_HEAD]),
                        k_v,
                    )
                    warm(qk[0:1, 0, 0:2, :])
                    return qk

                def qk_tree(i, qk):
                    t16 = peri.tile([128, L, HEADS, 16], bf16, tag="t16")
                    nc.vector.tensor_add(t16[:P], qk[:P, :, :, 0:16], qk[:P, :, :, 16:32])
                    t8 = peri.tile([128, L, HEADS, 8], bf16, tag="t8")
                    nc.vector.tensor_add(t8[:P], t16[:P, :, :, 0:8], t16[:P, :, :, 8:16])
                    warm(t8[0:1, 0, :, :])
                    t4 = peri.tile([128, L, HEADS, 4], bf16, tag="t4")
                    nc.vector.tensor_add(t4[:P], t8[:P, :, :, 0:4], t8[:P, :, :, 4:8])
                    t2 = peri.tile([128, L, HEADS, 2], bf16, tag="t2")
                    nc.vector.tensor_add(t2[:P], t4[:P, :, :, 0:2], t4[:P, :, :, 2:4])
                    nc.vector.tensor_add(
                        A[:P, i].unsqueeze(3),
                        t2[:P, :, :, 0:1], t2[:P, :, :, 1:2],
                    )

                # QK^T: gpsimd muls issued first so they overlap DVE's own slices
                A = work.tile([128, L, L, HEADS], f32, tag="A")
                qk_g = {}
                for i in QK_MUL_POOL_I:
                    qk_g[i] = qk_mul(i, nc.gpsimd, "qkg")
                dve_is = [i for i in range(L) if i not in QK_MUL_POOL_I]
                for i in dve_is:
                    qk_tree(i, qk_mul(i, nc.vector, "qkp"))
                for i in QK_MUL_POOL_I:
                    qk_tree(i, qk_g[i])

                # masked softmax over j (no max-subtraction)
                am = work.tile([128, L, L, HEADS], f32, tag="am")
                nc.vector.tensor_add(
                    am[:P], A[:P],
                    st["mbias"][:P].unsqueeze(1).unsqueeze(3).broadcast_to([P, L, L, HEADS]),
                )
                ee = work.tile([128, L, L, HEADS], bf16, tag="ee")
                nc.scalar.activation(
                    out=ee[:P], in_=am[:P], func=mybir.ActivationFunctionType.Exp,
                    scale=SCALE,
                )
                ssum = work.tile([128, L, HEADS], f32, tag="ssum")
                nc.vector.reduce_sum(
                    out=ssum[:P], in_=ee[:P].transpose([0, 1, 3, 2]), axis=mybir.AxisListType.X
                )
                warm(ee[0:1, 0, :, :])
                sinv = work.tile([128, L, HEADS], f32, tag="sinv")
                nc.vector.reciprocal(out=sinv[:P], in_=ssum[:P])
                pw = work.tile([128, L, L, HEADS], bf16, tag="pw")
                nc.vector.tensor_mul(
                    pw[:P], ee[:P],
                    sinv[:P].unsqueeze(2).broadcast_to([P, L, L, HEADS]),
                )
                st["pw"] = pw

            def stage_b(st):
                """attention-weighted V via bcast-mul + j-tree -> st['attout'].

                V rides in (d, m) order (host-permuted w_qkv columns), so the
                pw broadcast lands on a non-inner dim and both operands stay
                2x-packable on the DVE.  w_out rows are host-permuted to match.
                """
                P = st["P"]
                pw = st["pw"]
                qkv_bf = st["qkv_bf"]
                v_v = qkv_bf[:P, :, 2 * INNER:3 * INNER].rearrange("p l (d m) -> p l d m", m=HEADS)
                attout = work.tile([128, L, INNER], bf16, tag="attout")
                attout_v = attout[:P].rearrange("p i (d m) -> p i d m", m=HEADS)

                def av_mul(i, eng, tag):
                    av = peri.tile([128, L, DIM_HEAD, HEADS], bf16, tag=tag)
                    eng.tensor_mul(
                        av[:P],
                        v_v,
                        pw[:P, i, :, :].unsqueeze(2).broadcast_to([P, L, DIM_HEAD, HEADS]),
                    )
                    warm(av[0:1, 0, 0:2, :])
                    return av

                def av_tree(i, av):
                    t2j = peri.tile([128, 2, DIM_HEAD, HEADS], bf16, tag="t2j")
                    nc.vector.tensor_add(t2j[:P], av[:P, 0:2], av[:P, 2:4])
                    warm(t2j[0:1, 0, 0:2, :])
                    t1j = peri.tile([128, DIM_HEAD, HEADS], bf16, tag="t1j")
                    nc.vector.tensor_add(t1j[:P], t2j[:P, 0], t2j[:P, 1])
                    nc.vector.tensor_add(attout_v[:, i], t1j[:P], av[:P, 4])

                av_g = {}
                for i in AV_MUL_POOL_I:
                    av_g[i] = av_mul(i, nc.gpsimd, "avg")
                dve_is = [i for i in range(L) if i not in AV_MUL_POOL_I]
                for i in dve_is:
                    av_tree(i, av_mul(i, nc.vector, "avp"))
                for i in AV_MUL_POOL_I:
                    av_tree(i, av_g[i])
                st["attout"] = attout

            def back(st):
                """attout transpose + output projection + DMA out (PE/ACT heavy)."""
                P = st["P"]
                attout = st["attout"]
                aot = work.tile([128, 2 * L, 128], bf16, tag="aot")

                def apose(i):
                    pt = ps_t.tile([128, 2, 128], bf16, tag="pst")
                    for cc in range(2):
                        nc.tensor.transpose(
                            pt[:, cc, :P],
                            attout[:P, i, cc * 128:(cc + 1) * 128],
                            ident[:P, :P],
                        )
                    nc.scalar.copy(out=aot[:, i * 2:i * 2 + 2, :], in_=pt[:, :, :])

                apose(0)
                for i in range(L):
                    if i + 1 < L:
                        apose(i + 1)
                    po = ps_o.tile([128, C], f32, tag="pso")
                    nc.tensor.matmul(
                        out=po[:P],
                        lhsT=ones1[:, :P],
                        rhs=bout_sb,
                        start=True,
                        stop=False,
                        skip_group_check=True,
                    )
                    for cc in range(2):
                        nc.tensor.matmul(
                            out=po[:P],
                            lhsT=aot[:, i * 2 + cc, :P],
                            rhs=wout_sb[:, cc, :],
                            start=False,
                            stop=(cc == 1),
                            skip_group_check=True,
                        )
                    osb = peri.tile([128, C], f32, tag="osb")
                    nc.scalar.copy(out=osb[:P], in_=po[:P])
                    for (p0, b, h, w0, w1) in st["pieces"]:
                        nc.sync.dma_start(
                            out=out_d[b, i, h, w0:w1, :],
                            in_=osb[p0:p0 + (w1 - w0), :],
                        )

            # 3-deep software pipeline: per iteration t issue
            #   load(t)+proj(t)  DMA + PE qkv
            #   stage_a(t-1)     DVE qk/softmax
            #   stage_b(t-2)     DVE av
            #   back(t-2)        PE: aot transpose, out-proj; store
            sts = {}
            for t in range(NTILES + 2):
                if t < NTILES:
                    sts[t] = load(t)
                    proj(sts[t])
                if 0 <= t - 1 < NTILES:
                    stage_a(sts[t - 1])
                if 0 <= t - 2 < NTILES:
                    stage_b(sts[t - 2])
                    back(sts[t - 2])
                    del sts[t - 2]
    nc.finalize()  # Bacc.compile(): legalize multi-wait instructions, alloc regs
    return nc


def get_nc():
    if "nc" not in _cached:
        _cached["nc"] = _build_bass()
    return _cached["nc"]


def make_in_maps(x, mask, w_qkv, w_out, b_out):
    """Host-side shard + repack: x is transposed to [cc, c, b, h, l, w] and
    cast to bf16; the mask becomes an f32 additive logit bias."""
    import ml_dtypes

    x = np.asarray(x, dtype=np.float32)
    mask = np.asarray(mask)
    w_qkv = np.ascontiguousarray(np.asarray(w_qkv), dtype=np.float32)
    w_out = np.ascontiguousarray(np.asarray(w_out), dtype=np.float32)
    b_out = np.ascontiguousarray(np.asarray(b_out), dtype=np.float32)

    # permute V's output columns (m,d)->(d,m) and w_out's rows to match, so
    # the device-side pw broadcast is never on the innermost dim
    wv = w_qkv[:, 2 * INNER:].reshape(C, HEADS, DIM_HEAD).transpose(0, 2, 1).reshape(C, INNER)
    w_qkv = np.ascontiguousarray(np.concatenate([w_qkv[:, :2 * INNER], wv], axis=1))
    w_out = np.ascontiguousarray(
        w_out.reshape(HEADS, DIM_HEAD, C).transpose(1, 0, 2).reshape(INNER, C)
    )

    # [B, L, H, W, C] -> [C, B, H, L, W] -> [2, 128, B, H, L, W] bf16
    xT = np.transpose(x, (4, 0, 2, 1, 3)).astype(ml_dtypes.bfloat16)
    xT = np.ascontiguousarray(xT.reshape(2, 128, B, H, L, W))
    # [B, H, W, 1, L] -> f32 bias [B, H, W, L]
    mb = np.ascontiguousarray(
        np.where(mask[:, :, :, 0, :] != 0, 0.0, -1.0e4).astype(np.float32)
    )

    in_maps = []
    for k in range(NCORES):
        h0, h1 = k * HP, (k + 1) * HP
        in_maps.append({
            "xT": np.ascontiguousarray(xT[:, :, :, h0:h1]),
            "mbias": np.ascontiguousarray(mb[:, h0:h1]),
            "w_qkv": w_qkv,
            "w_out": w_out,
            "b_out": b_out,
        })
    return in_maps


def kernel(x, mask, w_qkv, w_out, b_out):
    from concourse.bass_utils import run_bass_kernel_spmd

    nc = get_nc()
    in_maps = make_in_maps(x, mask, w_qkv, w_out, b_out)
    res = run_bass_kernel_spmd(nc, in_maps, core_ids=list(range(NCORES)))
    out = np.concatenate([r["out"] for r in res.results], axis=2)
    return out
